# revision 5
# baseline (speedup 1.0000x reference)
"""Bahdanau attention kernel for Trainium2, 8 NeuronCores, data-parallel over batch.

Reference computation (per batch b):
    hq = query @ Wq_w.T + Wq_b          # [Q, A]
    hy = y @ Wy_w.T + Wy_b              # [Y, A]
    scores[q, y] = v_w . tanh(hq[q] + hy[y]) + v_b   # [Q, Y]
    att = softmax(scores, axis=y)       # [Q, Y]
    sim[q] = max_y scores[q, y]         # [1, Q]

Shapes: B=8, Q=256, Y=512, D=1024, A=256.

Kernel strategy (one batch per core):
  - Keep the A dim on SBUF partitions (2 tiles of 128).
  - hqT [a, q], hyT [a, y] computed via PE matmuls on transposed operands
    (PE transpose-mode for query/y/W transposes), float32r matmul dtype.
  - Per q: S[a, y] = hyT[a, y] + hqT'[a, q] via DVE tensor_scalar_add
    (per-partition scalar); batches of 16 q's are tanh'd in a single
    big ACT instruction (in-place, bf16).
  - Score dot: PE matmul with a sliding-window stationary ZV (v at
    column 127) so out[q, y] lands directly in a [128, 512] PSUM tile.
  - Softmax: DVE reduce_max -> ACT Exp(bias=-max, accum_out=sum) ->
    DVE reciprocal -> scale.  sim = max + v_b.
"""

import numpy as np

B, Q, Y, D, A = 8, 256, 512, 1024, 256
KT = D // 128   # k tiles in contraction dim
AT = A // 128   # a tiles
QB = Q // 128   # q blocks
YTILES = Y // 128
QTILES = Q // 128
QC = 16         # q's per ACT chunk
NCH = 128 // QC

_cached = None


def _build():
    import concourse.bass as bass
    import concourse.tile as tile
    from concourse import bacc, mybir
    from concourse import masks

    f32 = mybir.dt.float32
    f32r = mybir.dt.float32r
    bf16 = mybir.dt.bfloat16
    ts = bass.ts
    AF = mybir.ActivationFunctionType

    nc = bacc.Bacc("TRN2", target_bir_lowering=False, debug=False)

    query_ext = nc.dram_tensor("query", [Q, D], f32, kind="ExternalInput")
    y_ext = nc.dram_tensor("y", [Y, D], f32, kind="ExternalInput")
    wq_ext = nc.dram_tensor("Wq_w", [A, D], f32, kind="ExternalInput")
    wqb_ext = nc.dram_tensor("Wq_b", [A], f32, kind="ExternalInput")
    wy_ext = nc.dram_tensor("Wy_w", [A, D], f32, kind="ExternalInput")
    wyb_ext = nc.dram_tensor("Wy_b", [A], f32, kind="ExternalInput")
    v_ext = nc.dram_tensor("v_w", [1, A], f32, kind="ExternalInput")
    vb_ext = nc.dram_tensor("v_b", [1], f32, kind="ExternalInput")
    att_ext = nc.dram_tensor("att", [Q, Y], f32, kind="ExternalOutput")
    sim_ext = nc.dram_tensor("sim", [1, Q], f32, kind="ExternalOutput")

    with tile.TileContext(nc) as tc:
        from contextlib import ExitStack
        ctx = ExitStack()
        with ctx:
            consts = ctx.enter_context(tc.tile_pool(name="consts", bufs=1))
            nat_pool = ctx.enter_context(tc.tile_pool(name="nat", bufs=4))
            tr_sb = ctx.enter_context(tc.tile_pool(name="tr_sb", bufs=1))
            s_pool = ctx.enter_context(tc.tile_pool(name="s", bufs=3))
            soft_pool = ctx.enter_context(tc.tile_pool(name="soft", bufs=2))
            small = ctx.enter_context(tc.tile_pool(name="small", bufs=2))
            psum_tr = ctx.enter_context(
                tc.tile_pool(name="ps_tr", bufs=2, space="PSUM"))
            psum_proj = ctx.enter_context(
                tc.tile_pool(name="ps_proj", bufs=1, space="PSUM"))
            psum_sc = ctx.enter_context(
                tc.tile_pool(name="ps_sc", bufs=2, space="PSUM"))

            # ---- constants ----
            ident = consts.tile([128, 128], f32)
            masks.make_identity(nc, ident[:])
            ones_row = consts.tile([1, 128], f32)
            nc.gpsimd.memset(ones_row[:], 1.0)

            # per-partition vectors
            bq_sb = [consts.tile([128, 1], f32, name=f"bq{t}") for t in range(AT)]
            by_sb = [consts.tile([128, 1], f32, name=f"by{t}") for t in range(AT)]
            v_sb = [consts.tile([128, 1], f32, name=f"v{t}") for t in range(AT)]
            vb_sb = consts.tile([1, 1], f32)
            for t in range(AT):
                nc.sync.dma_start(bq_sb[t][:], wqb_ext.ap()[ts(t, 128)].unsqueeze(1))
                nc.sync.dma_start(by_sb[t][:], wyb_ext.ap()[ts(t, 128)].unsqueeze(1))
                nc.sync.dma_start(v_sb[t][:], v_ext.ap()[0, ts(t, 128)].unsqueeze(1))
            nc.sync.dma_start(vb_sb[:], vb_ext.ap().unsqueeze(0))

            # combined bias (Wq_b + Wy_b) per a-tile
            cb = [consts.tile([128, 1], f32, name=f"cb{t}") for t in range(AT)]
            for t in range(AT):
                nc.vector.tensor_add(cb[t][:], bq_sb[t][:], by_sb[t][:])

            # vb broadcast to 128 partitions via K=1 matmul with ones
            ps_vb = psum_tr.tile([128, 1], f32, tag="tr")
            nc.tensor.matmul(ps_vb[:], ones_row[:], vb_sb[:], start=True, stop=True)
            vb_bc = consts.tile([128, 1], f32)
            nc.vector.tensor_copy(vb_bc[:], ps_vb[:])

            # ZV sliding-window stationaries: v at column 127, zeros elsewhere
            zv = [consts.tile([128, 256], bf16, name=f"zv{t}") for t in range(AT)]
            for t in range(AT):
                nc.gpsimd.memset(zv[t][:], 0.0)
                nc.vector.tensor_copy(zv[t][:, 127:128], v_sb[t][:])

            # ---- load + transpose query / y / weights ----
            # transposed f32 buffers: per k-tile
            qT = [tr_sb.tile([128, Q], bf16, name=f"qT{k}") for k in range(KT)]
            yT = [tr_sb.tile([128, Y], bf16, name=f"yT{k}") for k in range(KT)]
            wqT = [tr_sb.tile([128, A], bf16, name=f"wqT{k}") for k in range(KT)]
            wyT = [tr_sb.tile([128, A], bf16, name=f"wyT{k}") for k in range(KT)]

            def load_and_transpose(ext, nrows, dst):
                # ext: [nrows*128, D] DRAM; dst[k][:, ts(i,128)] = block.T
                for i in range(nrows):
                    nat = nat_pool.tile([128, D], f32, tag="nat")
                    nc.sync.dma_start(nat[:], ext.ap()[ts(i, 128), :])
                    for k in range(KT):
                        ps = psum_tr.tile([128, 128], f32, tag="tr")
                        nc.tensor.transpose(ps[:], nat[:, ts(k, 128)], ident[:])
                        nc.vector.tensor_copy(dst[k][:, ts(i, 128)], ps[:])

            load_and_transpose(query_ext, QTILES, qT)
            load_and_transpose(wq_ext, AT, wqT)
            load_and_transpose(wy_ext, AT, wyT)
            load_and_transpose(y_ext, YTILES, yT)

            # ---- projections ----
            # hqT'[a, q] = sum_d WqT[d, a] * qT[d, q] + (bq + by)   (f32)
            hqT = [tr_sb.tile([128, Q], f32, name=f"hqT{t}") for t in range(AT)]
            for t in range(AT):
                ps = psum_proj.tile([128, Q], f32, tag="hq")
                for k in range(KT):
                    nc.tensor.matmul(ps[:], wqT[k][:, ts(t, 128)], qT[k][:],
                                     start=(k == 0), stop=(k == KT - 1))
                nc.vector.tensor_scalar_add(hqT[t][:], ps[:], cb[t][:, 0:1])

            # hyT[a, y] (bf16, no bias)
            hyT = [tr_sb.tile([128, Y], bf16, name=f"hyT{t}") for t in range(AT)]
            for t in range(AT):
                ps = psum_proj.tile([128, Y], f32, tag="hy")
                for k in range(KT):
                    nc.tensor.matmul(ps[:], wyT[k][:, ts(t, 128)], yT[k][:],
                                     start=(k == 0), stop=(k == KT - 1))
                nc.vector.tensor_copy(hyT[t][:], ps[:])

            # ---- main loop: tanh + score dot, then softmax per q-block ----
            for qb in range(QB):
                ps_scores = psum_sc.tile([128, Y], f32, tag="scores")
                for ch in range(NCH):
                    for t in range(AT):
                        S = s_pool.tile([128, QC * Y], bf16, tag="S")
                        for j in range(QC):
                            q = qb * 128 + ch * QC + j
                            nc.vector.tensor_scalar_add(
                                S[:, ts(j, Y)], hyT[t][:], hqT[t][:, q:q + 1])
                        nc.scalar.activation(S[:], S[:], AF.Tanh)
                        for j in range(QC):
                            ql = ch * QC + j
                            first = (ch == 0 and t == 0 and j == 0)
                            last = (ch == NCH - 1 and t == AT - 1 and j == QC - 1)
                            nc.tensor.matmul(
                                ps_scores[:],
                                zv[t][:, 127 - ql:255 - ql],
                                S[:, ts(j, Y)],
                                start=first, stop=last)

                # softmax over y (free dim) for this 128-q block
                mx = small.tile([128, 1], f32, tag="mx")
                nc.vector.reduce_max(mx[:], ps_scores[:], axis=mybir.AxisListType.X)
                nmx = small.tile([128, 1], f32, tag="nmx")
                nc.vector.tensor_scalar_mul(nmx[:], mx[:], -1.0)
                e_sb = soft_pool.tile([128, Y], f32, tag="e")
                sum_e = small.tile([128, 1], f32, tag="sum")
                nc.scalar.activation(e_sb[:], ps_scores[:], AF.Exp,
                                     bias=nmx[:, 0:1], accum_out=sum_e[:, 0:1])
                rinv = small.tile([128, 1], f32, tag="rinv")
                nc.vector.reciprocal(rinv[:], sum_e[:])
                nc.vector.tensor_scalar_mul(e_sb[:], e_sb[:], rinv[:, 0:1])
                nc.sync.dma_start(att_ext.ap()[ts(qb, 128), :], e_sb[:])

                sim_sb = small.tile([128, 1], f32, tag="sim")
                nc.vector.tensor_add(sim_sb[:], mx[:], vb_bc[:])
                nc.sync.dma_start(sim_ext.ap()[0:1, ts(qb, 128)], sim_sb[:])

    nc.compile()
    return nc


def _get_nc():
    global _cached
    if _cached is None:
        _cached = _build()
    return _cached


def kernel(query, y, Wq_w, Wq_b, Wy_w, Wy_b, v_w, v_b):
    from concourse.bass_utils import run_bass_kernel_spmd

    nc = _get_nc()
    in_maps = []
    for b in range(B):
        in_maps.append({
            "query": np.ascontiguousarray(query[b], dtype=np.float32),
            "y": np.ascontiguousarray(y[b], dtype=np.float32),
            "Wq_w": np.ascontiguousarray(Wq_w, dtype=np.float32),
            "Wq_b": np.ascontiguousarray(Wq_b, dtype=np.float32),
            "Wy_w": np.ascontiguousarray(Wy_w, dtype=np.float32),
            "Wy_b": np.ascontiguousarray(Wy_b, dtype=np.float32),
            "v_w": np.ascontiguousarray(v_w, dtype=np.float32),
            "v_b": np.ascontiguousarray(v_b, dtype=np.float32),
        })
    res = run_bass_kernel_spmd(nc, in_maps, core_ids=list(range(B)))
    att = np.stack([res.results[b]["att"] for b in range(B)])
    sim = np.stack([res.results[b]["sim"] for b in range(B)])
    return att.astype(np.float32), sim.astype(np.float32)
